# revision 3
# baseline (speedup 1.0000x reference)
"""MoE ExpertBlock (16 experts, top-4, SwiGLU) on 8 Trainium2 NeuronCores.

Strategy (expert-parallel, per sharding hint):
  - Host: router (x @ router_w.T + bias -> softmax -> top-4) and token
    dispatch. This is ~0.07% of the model FLOPs.
  - Device: each of the 8 cores runs the SwiGLU FFN for 2 experts over the
    tokens routed to them (gathered + padded to a uniform capacity C).
    Feature-major layout ([H, C] activations, features on partitions) so the
    whole FFN chain needs zero on-device transposes; matmuls run in float32r
    (full PE rate for N>=256, ~1e-4 matmul rel-err).
  - Host: scatter-add the weighted per-expert outputs back (top-4 combine).

Device compute per core: 2 experts x 3 matmuls x [C,2048]x[2048,1792]-class
GEMMs ~ 25 GFLOP, vs 4x more for the dense-all-experts reference.
"""

import sys

sys.path.insert(0, "/opt/trn_rl_repo")

from contextlib import ExitStack

import numpy as np

import concourse.bacc as bacc
import concourse.mybir as mybir
import concourse.tile as tile
from concourse.bass_utils import run_bass_kernel_spmd

B, S, H, I, E, TOPK = 2, 1024, 2048, 1792, 16, 4
T = B * S
NCORES = 8
EPC = E // NCORES  # experts per core
KH = H // 128  # 16 k-tiles over hidden dim
KI = I // 128  # 14 tiles over intermediate dim

F32 = mybir.dt.float32
F32R = mybir.dt.float32r
MULT = mybir.AluOpType.mult
SILU = mybir.ActivationFunctionType.Silu


def _slices(C):
    """Split C into contiguous chunks, each <=512 and >=256 (PSUM-bank sized,
    full-rate fp32r). C must be a multiple of 64 and >= 256."""
    n = -(-C // 512)
    out = []
    rem = C
    for i in range(n):
        s = min(512, -(-rem // (n - i) // 64) * 64)
        out.append(s)
        rem -= s
    assert rem == 0 and all(256 <= s <= 512 for s in out), (C, out)
    return out


def _route(x, router_w, expert_bias):
    """Host router: top-4 expert ids + renormalized weights per token."""
    xf = x.reshape(T, H).astype(np.float32)
    logits = xf @ router_w.T.astype(np.float32) + expert_bias.astype(np.float32)
    # top-4 by logit (same order as softmax); stable sort matches jax top_k ties
    idx = np.argsort(-logits, axis=-1, kind="stable")[:, :TOPK]
    l4 = np.take_along_axis(logits, idx, axis=-1)
    w = np.exp(l4 - l4.max(-1, keepdims=True))
    w = w / w.sum(-1, keepdims=True)
    return idx.astype(np.int32), w.astype(np.float32)


def _build_nc(C, slices, repeat=1):
    """Build the SPMD Bass program: 2 experts/core, SwiGLU over [H,C] tokens."""
    nc = bacc.Bacc(
        "TRN2",
        target_bir_lowering=False,
        debug=False,
        enable_asserts=True,
        num_devices=NCORES,
    )
    xt_d = nc.dram_tensor("xt", [EPC, H, C], F32R, kind="ExternalInput").ap()
    wg_d = nc.dram_tensor("wg", [EPC, H, I], F32R, kind="ExternalInput").ap()
    wu_d = nc.dram_tensor("wu", [EPC, H, I], F32R, kind="ExternalInput").ap()
    wd_d = nc.dram_tensor("wd", [EPC, I, H], F32R, kind="ExternalInput").ap()
    yt_d = nc.dram_tensor("yt", [EPC, H, C], F32, kind="ExternalOutput").ap()

    with tile.TileContext(nc) as tc, ExitStack() as ctx:
        xpool = ctx.enter_context(tc.tile_pool(name="x", bufs=2 * KH))
        apool = ctx.enter_context(tc.tile_pool(name="a", bufs=KI))
        wpool = ctx.enter_context(tc.tile_pool(name="w", bufs=4))
        tpool = ctx.enter_context(tc.tile_pool(name="t", bufs=2))
        ypool = ctx.enter_context(tc.tile_pool(name="y", bufs=4))
        ppool = ctx.enter_context(tc.tile_pool(name="p", bufs=2, space="PSUM"))

        def mm(psum, w_tile, rhs, k, klast):
            nc.tensor.matmul(
                psum[:],
                w_tile,
                rhs,
                start=(k == 0),
                stop=(k == klast),
            )

        for _ in range(repeat):
            for j in range(EPC):
                # activations X^T for this expert: 16 x [128, C]
                xs = []
                for k in range(KH):
                    t = xpool.tile([128, C], F32R, tag="xk", name="xk")
                    nc.sync.dma_start(t[:], xt_d[j, k * 128 : (k + 1) * 128, :])
                    xs.append(t)
                at = [apool.tile([128, C], F32R, tag="ak", name="ak") for _ in range(KI)]

                # ---- gate/up + SwiGLU, two I-tiles (m) at a time ----
                for mg in range(0, KI, 2):
                    pg = [
                        [ppool.tile([128, s], F32, tag=f"p{mi}{si}", name=f"p{mi}{si}")
                         for si, s in enumerate(slices)]
                        for mi in range(2)
                    ]
                    for k in range(KH):
                        w = wpool.tile([128, 256], F32R, tag="wg", name="wg")
                        nc.sync.dma_start(
                            w[:],
                            wg_d[j, k * 128 : (k + 1) * 128,
                                 mg * 128 : (mg + 2) * 128],
                        )
                        for mi in range(2):
                            off = 0
                            for si, s in enumerate(slices):
                                mm(pg[mi][si],
                                   w[:, mi * 128 : (mi + 1) * 128],
                                   xs[k][:, off : off + s], k, KH - 1)
                                off += s
                    # silu(gate) -> sbuf tmp
                    tg = [tpool.tile([128, C], F32, tag="tg", name="tg") for _ in range(2)]
                    for mi in range(2):
                        off = 0
                        for si, s in enumerate(slices):
                            nc.scalar.activation(
                                tg[mi][:, off : off + s], pg[mi][si][:], SILU)
                            off += s
                    # up projection into fresh psum (same tags -> same banks)
                    pu = [
                        [ppool.tile([128, s], F32, tag=f"p{mi}{si}", name=f"p{mi}{si}")
                         for si, s in enumerate(slices)]
                        for mi in range(2)
                    ]
                    for k in range(KH):
                        w = wpool.tile([128, 256], F32R, tag="wu", name="wu")
                        nc.sync.dma_start(
                            w[:],
                            wu_d[j, k * 128 : (k + 1) * 128,
                                 mg * 128 : (mg + 2) * 128],
                        )
                        for mi in range(2):
                            off = 0
                            for si, s in enumerate(slices):
                                mm(pu[mi][si],
                                   w[:, mi * 128 : (mi + 1) * 128],
                                   xs[k][:, off : off + s], k, KH - 1)
                                off += s
                    # act = silu(g) * u
                    for mi in range(2):
                        off = 0
                        for si, s in enumerate(slices):
                            nc.vector.tensor_tensor(
                                at[mg + mi][:, off : off + s],
                                tg[mi][:, off : off + s],
                                pu[mi][si][:],
                                MULT,
                            )
                            off += s

                # ---- down projection, two H-tiles at a time ----
                for hg in range(0, KH, 2):
                    py = [
                        [ppool.tile([128, s], F32, tag=f"p{mi}{si}", name=f"p{mi}{si}")
                         for si, s in enumerate(slices)]
                        for mi in range(2)
                    ]
                    for ki in range(KI):
                        w = wpool.tile([128, 256], F32R, tag="wd", name="wd")
                        nc.sync.dma_start(
                            w[:],
                            wd_d[j, ki * 128 : (ki + 1) * 128,
                                 hg * 128 : (hg + 2) * 128],
                        )
                        for mi in range(2):
                            off = 0
                            for si, s in enumerate(slices):
                                mm(py[mi][si],
                                   w[:, mi * 128 : (mi + 1) * 128],
                                   at[ki][:, off : off + s], ki, KI - 1)
                                off += s
                    for mi in range(2):
                        yo = ypool.tile([128, C], F32, tag="yo", name="yo")
                        off = 0
                        for si, s in enumerate(slices):
                            nc.vector.tensor_copy(
                                yo[:, off : off + s], py[mi][si][:])
                            off += s
                        nc.sync.dma_start(
                            yt_d[j, (hg + mi) * 128 : (hg + mi + 1) * 128, :],
                            yo[:],
                        )

    nc.compile()
    return nc


def _prep(x, gate_proj, up_proj, down_proj, idx, C):
    """Gather per-expert token sets into per-core device inputs."""
    xf = np.ascontiguousarray(x.reshape(T, H).astype(np.float32))
    tok = [np.nonzero((idx == e).any(-1))[0] for e in range(E)]
    in_maps = []
    for c in range(NCORES):
        xt = np.zeros((EPC, H, C), np.float32)
        for j in range(EPC):
            e = c * EPC + j
            te = tok[e]
            xt[j, :, : len(te)] = xf[te].T
        es = slice(c * EPC, (c + 1) * EPC)
        in_maps.append(
            {
                "xt": xt,
                "wg": np.ascontiguousarray(gate_proj[es], dtype=np.float32),
                "wu": np.ascontiguousarray(up_proj[es], dtype=np.float32),
                "wd": np.ascontiguousarray(down_proj[es], dtype=np.float32),
            }
        )
    return in_maps, tok


def _combine(results, tok, idx, wts):
    """Weighted scatter-add of per-expert outputs back to [T, H]."""
    out = np.zeros((T, H), np.float64)
    for e in range(E):
        c, j = divmod(e, EPC)
        yt = results[c]["yt"][j]  # [H, C]
        te = tok[e]
        # weight of expert e for each of its tokens
        k = np.argmax(idx[te] == e, axis=-1)
        w = wts[te, k]
        out[te] += yt[:, : len(te)].T.astype(np.float64) * w[:, None]
    return out.astype(np.float32).reshape(B, S, H)


def kernel(x, router_w, expert_bias, gate_proj, up_proj, down_proj):
    x = np.asarray(x)
    idx, wts = _route(np.asarray(x), np.asarray(router_w), np.asarray(expert_bias))
    counts = np.bincount(idx.ravel(), minlength=E)
    C = max(256, int(-(-counts.max() // 64) * 64))
    nc = _build_nc(C, _slices(C))
    in_maps, tok = _prep(
        x, np.asarray(gate_proj), np.asarray(up_proj), np.asarray(down_proj), idx, C
    )
    res = run_bass_kernel_spmd(nc, in_maps, list(range(NCORES)))
    return _combine(res.results, tok, idx, wts)


# revision 11
# speedup vs baseline: 649.5095x; 649.5095x over previous
"""MoE ExpertBlock (16 experts, top-4, SwiGLU) on 8 Trainium2 NeuronCores.

Strategy (expert-parallel, per sharding hint):
  - Host: router (x @ router_w.T + bias -> softmax -> top-4) and token
    dispatch. This is ~0.07% of the model FLOPs.
  - Device: each of the 8 cores runs the SwiGLU FFN for 2 experts over the
    tokens routed to them. Experts are load-sorted: slot 0 = the 8 largest
    token counts (capacity CA), slot 1 = the 8 smallest (CB; when CB <= 512
    each PSUM tile is a single full-rate N=512 matmul).
    Feature-major layout ([H, C] activations, features on partitions) so the
    whole FFN chain needs zero on-device transposes; matmuls run in float32r
    (full PE rate for N>=256, ~2.6e-4 end-to-end rel-err vs the fp32
    reference). Weights stream from HBM in batched 3D-AP DMAs (~1-2 MB each,
    4-deep prefetch) -- small per-(k,m) weight DMAs were queue-bound.
  - Host: scatter-add the weighted per-expert outputs back (top-4 combine).

Device compute per core: 2 experts x 3 matmuls x [C,2048]x[2048,1792]-class
GEMMs ~ 24 GFLOP, vs 4x more for the dense-all-experts reference.
Measured ~420-460 us/core HW exec vs ~305 us pure PE-streaming bound.
"""

import sys

sys.path.insert(0, "/opt/trn_rl_repo")

from contextlib import ExitStack

import numpy as np

import concourse.bacc as bacc
import concourse.mybir as mybir
import concourse.tile as tile
from concourse.bass_utils import run_bass_kernel_spmd

B, S, H, I, E, TOPK = 2, 1024, 2048, 1792, 16, 4
T = B * S
NCORES = 8
EPC = E // NCORES  # experts per core
KH = H // 128  # 16 k-tiles over hidden dim
KI = I // 128  # 14 tiles over intermediate dim

F32 = mybir.dt.float32
F32R = mybir.dt.float32r
BF16 = mybir.dt.bfloat16
MMDT = F32R  # matmul operand dtype
MULT = mybir.AluOpType.mult
SILU = mybir.ActivationFunctionType.Silu


def _slices(C):
    """Split C into contiguous chunks, each <=512 and >=256 (PSUM-bank sized,
    full-rate fp32r). C must be a multiple of 64 and >= 256."""
    n = -(-C // 512)
    out = []
    rem = C
    for i in range(n):
        s = min(512, -(-rem // (n - i) // 64) * 64)
        out.append(s)
        rem -= s
    assert rem == 0 and all(256 <= s <= 512 for s in out), (C, out)
    return out


def _route(x, router_w, expert_bias):
    """Host router: top-4 expert ids + renormalized weights per token."""
    xf = x.reshape(T, H).astype(np.float32)
    logits = xf @ router_w.T.astype(np.float32) + expert_bias.astype(np.float32)
    # top-4 by logit (same order as softmax); stable sort matches jax top_k ties
    idx = np.argsort(-logits, axis=-1, kind="stable")[:, :TOPK]
    l4 = np.take_along_axis(logits, idx, axis=-1)
    w = np.exp(l4 - l4.max(-1, keepdims=True))
    w = w / w.sum(-1, keepdims=True)
    return idx.astype(np.int32), w.astype(np.float32)


def _build_nc(Cs, slices_list, repeat=1, mmdt=None):
    """Build the SPMD Bass program: 2 experts/core, SwiGLU over [H,C] tokens.

    Cs/slices_list: per-slot token capacity and PSUM n-slicing. Slot 0 holds
    the big-count experts, slot 1 the small ones (fewer/larger matmuls)."""
    mmdt = mmdt or MMDT
    CA = Cs[0]
    nc = bacc.Bacc(
        "TRN2",
        target_bir_lowering=False,
        debug=False,
        enable_asserts=True,
        num_devices=NCORES,
    )
    xt_d = nc.dram_tensor("xt", [EPC, H, CA], mmdt, kind="ExternalInput").ap()
    wg_d = nc.dram_tensor("wg", [EPC, H, I], mmdt, kind="ExternalInput").ap()
    wu_d = nc.dram_tensor("wu", [EPC, H, I], mmdt, kind="ExternalInput").ap()
    wd_d = nc.dram_tensor("wd", [EPC, I, H], mmdt, kind="ExternalInput").ap()
    yt_d = nc.dram_tensor("yt", [EPC, H, CA], F32, kind="ExternalOutput").ap()

    # k-tiles per batched weight DMA (SBUF budget: bf16 fits full-k blocks)
    WKB = KH if mmdt == BF16 else 8
    WKBD = KI if mmdt == BF16 else 7

    with tile.TileContext(nc) as tc, ExitStack() as ctx:
        xpool = ctx.enter_context(tc.tile_pool(name="x", bufs=2 if mmdt == BF16 else 1))
        apool = ctx.enter_context(tc.tile_pool(name="a", bufs=KI))
        wpool = ctx.enter_context(tc.tile_pool(name="w", bufs=4))
        tpool = ctx.enter_context(tc.tile_pool(name="t", bufs=4))
        ypool = ctx.enter_context(tc.tile_pool(name="y", bufs=3))
        ppool = ctx.enter_context(tc.tile_pool(name="p", bufs=2, space="PSUM"))

        def load_w(src_j, k0, nk, col0, tag):
            """One DMA: weight block [128, nk(k-tiles), 256(2 m-tiles)]."""
            t = wpool.tile([128, nk * 256], mmdt, tag=tag, name=tag)
            nc.sync.dma_start(
                t[:].rearrange("p (k c) -> p k c", c=256),
                src_j.rearrange("(k p) c -> p k c", p=128)[
                    :, k0 : k0 + nk, col0 : col0 + 256
                ],
            )
            return t

        def mmacc(psums, wt, k0, nk, rhs_of_k, ktot, slices):
            for kk in range(nk):
                k = k0 + kk
                for mi in range(2):
                    off = 0
                    for si, s in enumerate(slices):
                        nc.tensor.matmul(
                            psums[mi][si][:],
                            wt[:, (kk * 2 + mi) * 128 : (kk * 2 + mi + 1) * 128],
                            rhs_of_k(k)[:, off : off + s],
                            start=(k == 0),
                            stop=(k == ktot - 1),
                        )
                        off += s

        def psum_pair(slices):
            return [
                [ppool.tile([128, s], F32, tag=f"p{mi}{si}", name=f"p{mi}{si}")
                 for si, s in enumerate(slices)]
                for mi in range(2)
            ]

        def body():
            for j in range(EPC):
                C, slices = Cs[j], slices_list[j]
                # all activations X^T for this expert in one DMA
                xs = xpool.tile([128, KH * C], mmdt, tag="xk", name="xk")
                nc.sync.dma_start(
                    xs[:].rearrange("p (k c) -> p k c", c=C),
                    xt_d[j].rearrange("(k p) c -> p k c", p=128)[:, :, :C],
                )

                def xk(k):
                    return xs[:, k * C : (k + 1) * C]

                at = [apool.tile([128, C], mmdt, tag="ak", name="ak")
                      for _ in range(KI)]

                def atk(k):
                    return at[k][:]

                # ---- gate/up + SwiGLU, two I-tiles (m) at a time ----
                for mg in range(0, KI, 2):
                    col0 = mg * 128
                    pg = psum_pair(slices)
                    for k0 in range(0, KH, WKB):
                        wt = load_w(wg_d[j], k0, min(WKB, KH - k0), col0, "wg")
                        mmacc(pg, wt, k0, min(WKB, KH - k0), xk, KH, slices)
                    tg = [tpool.tile([128, C], F32, tag="tg", name="tg")
                          for _ in range(2)]
                    for mi in range(2):
                        off = 0
                        for si, s in enumerate(slices):
                            nc.scalar.activation(
                                tg[mi][:, off : off + s], pg[mi][si][:], SILU)
                            off += s
                    pu = psum_pair(slices)
                    for k0 in range(0, KH, WKB):
                        wt = load_w(wu_d[j], k0, min(WKB, KH - k0), col0, "wu")
                        mmacc(pu, wt, k0, min(WKB, KH - k0), xk, KH, slices)
                    # act = silu(g) * u
                    for mi in range(2):
                        off = 0
                        for si, s in enumerate(slices):
                            nc.vector.tensor_tensor(
                                at[mg + mi][:, off : off + s],
                                tg[mi][:, off : off + s],
                                pu[mi][si][:],
                                MULT,
                            )
                            off += s

                # ---- down projection, two H-tiles at a time ----
                for hg in range(0, KH, 2):
                    col0 = hg * 128
                    py = psum_pair(slices)
                    for k0 in range(0, KI, WKBD):
                        wt = load_w(wd_d[j], k0, min(WKBD, KI - k0), col0, "wd")
                        mmacc(py, wt, k0, min(WKBD, KI - k0), atk, KI, slices)
                    yo = ypool.tile([128, 2 * C], F32, tag="yo", name="yo")
                    for mi in range(2):
                        off = 0
                        for si, s in enumerate(slices):
                            nc.vector.tensor_copy(
                                yo[:, mi * C + off : mi * C + off + s],
                                py[mi][si][:])
                            off += s
                    nc.sync.dma_start(
                        yt_d[j].rearrange("(g p) c -> p g c", p=128)[
                            :, hg : hg + 2, :C],
                        yo[:].rearrange("p (g c) -> p g c", c=C),
                    )

        if repeat > 1:
            # HW loop used only by the timing harness: repeats the identical
            # body so HW exec time dominates the per-call dispatch overhead.
            with tc.For_i(0, repeat, 1):
                body()
        else:
            body()

    nc.compile()
    return nc


def _np_dt(mmdt):
    if mmdt == BF16:
        import ml_dtypes

        return ml_dtypes.bfloat16
    return np.float32


def _plan(counts):
    """Assign experts to (core, slot): slot 0 = 8 largest counts, slot 1 = 8
    smallest. Returns expert order and per-slot capacities."""
    order = np.argsort(-counts, kind="stable")
    caps = []
    for j in range(EPC):
        grp = order[j * NCORES : (j + 1) * NCORES]
        caps.append(max(256, int(-(-counts[grp].max() // 64) * 64)))
    return order, caps


def _prep(x, gate_proj, up_proj, down_proj, idx, order, caps, mmdt=None):
    """Gather per-expert token sets into per-core device inputs."""
    ndt = _np_dt(mmdt or MMDT)
    CA = caps[0]
    xf = np.ascontiguousarray(x.reshape(T, H).astype(np.float32))
    tok = [np.nonzero((idx == e).any(-1))[0] for e in range(E)]
    in_maps = []
    for c in range(NCORES):
        xt = np.zeros((EPC, H, CA), ndt)
        es = [int(order[j * NCORES + c]) for j in range(EPC)]
        for j, e in enumerate(es):
            te = tok[e]
            xt[j, :, : len(te)] = xf[te].T.astype(ndt)
        in_maps.append(
            {
                "xt": xt,
                "wg": np.ascontiguousarray(gate_proj[es]).astype(ndt),
                "wu": np.ascontiguousarray(up_proj[es]).astype(ndt),
                "wd": np.ascontiguousarray(down_proj[es]).astype(ndt),
            }
        )
    return in_maps, tok


def _combine(results, tok, idx, wts, order):
    """Weighted scatter-add of per-expert outputs back to [T, H]."""
    out = np.zeros((T, H), np.float64)
    for r in range(E):
        e = int(order[r])
        j, c = divmod(r, NCORES)
        yt = results[c]["yt"][j]  # [H, CA]
        te = tok[e]
        k = np.argmax(idx[te] == e, axis=-1)
        w = wts[te, k]
        out[te] += yt[:, : len(te)].T.astype(np.float64) * w[:, None]
    return out.astype(np.float32).reshape(B, S, H)


def kernel(x, router_w, expert_bias, gate_proj, up_proj, down_proj):
    x = np.asarray(x)
    idx, wts = _route(np.asarray(x), np.asarray(router_w), np.asarray(expert_bias))
    counts = np.bincount(idx.ravel(), minlength=E)
    order, caps = _plan(counts)
    nc = _build_nc(caps, [_slices(c) for c in caps])
    in_maps, tok = _prep(
        x, np.asarray(gate_proj), np.asarray(up_proj), np.asarray(down_proj),
        idx, order, caps,
    )
    res = run_bass_kernel_spmd(nc, in_maps, list(range(NCORES)))
    return _combine(res.results, tok, idx, wts, order)


# revision 17
# speedup vs baseline: 659.5232x; 1.0154x over previous
"""MoE ExpertBlock (16 experts, top-4, SwiGLU) on 8 Trainium2 NeuronCores.

Strategy (expert-parallel, per sharding hint):
  - Host: router (x @ router_w.T + bias -> softmax -> top-4) and token
    dispatch. This is ~0.07% of the model FLOPs.
  - Device: each of the 8 cores runs the SwiGLU FFN for 2 experts over the
    tokens routed to them. Experts are load-sorted: slot 0 = the 8 largest
    token counts (capacity CA), slot 1 = the 8 smallest (CB; when CB <= 512
    each PSUM tile is a single full-rate N=512 matmul).
    Feature-major layout ([H, C] activations, features on partitions) so the
    whole FFN chain needs zero on-device transposes; matmuls run in float32r
    (full PE rate for N>=256, ~2.6e-4 end-to-end rel-err vs the fp32
    reference). Weights stream from HBM in batched 3D-AP DMAs (~1-2 MB each,
    4-deep prefetch) -- small per-(k,m) weight DMAs were queue-bound.
  - Host: scatter-add the weighted per-expert outputs back (top-4 combine).

Device compute per core: 2 experts x 3 matmuls x [C,2048]x[2048,1792]-class
GEMMs ~ 24 GFLOP, vs 4x more for the dense-all-experts reference.
Measured ~420-460 us/core HW exec vs ~305 us pure PE-streaming bound.
"""

import sys

sys.path.insert(0, "/opt/trn_rl_repo")

from contextlib import ExitStack

import numpy as np

import concourse.bacc as bacc
import concourse.mybir as mybir
import concourse.tile as tile
from concourse.bass_utils import run_bass_kernel_spmd

B, S, H, I, E, TOPK = 2, 1024, 2048, 1792, 16, 4
T = B * S
NCORES = 8
EPC = E // NCORES  # experts per core
KH = H // 128  # 16 k-tiles over hidden dim
KI = I // 128  # 14 tiles over intermediate dim

F32 = mybir.dt.float32
F32R = mybir.dt.float32r
BF16 = mybir.dt.bfloat16
MMDT = F32R  # matmul operand dtype
MULT = mybir.AluOpType.mult
SILU = mybir.ActivationFunctionType.Silu


def _slices(C):
    """Split C into contiguous chunks, each <=512 and >=256 (PSUM-bank sized,
    full-rate fp32r). C must be a multiple of 64 and >= 256."""
    n = -(-C // 512)
    out = []
    rem = C
    for i in range(n):
        s = min(512, -(-rem // (n - i) // 64) * 64)
        out.append(s)
        rem -= s
    assert rem == 0 and all(256 <= s <= 512 for s in out), (C, out)
    return out


def _route(x, router_w, expert_bias):
    """Host router: top-4 expert ids + renormalized weights per token."""
    xf = x.reshape(T, H).astype(np.float32)
    logits = xf @ router_w.T.astype(np.float32) + expert_bias.astype(np.float32)
    # top-4 by logit (same order as softmax); stable sort matches jax top_k ties
    idx = np.argsort(-logits, axis=-1, kind="stable")[:, :TOPK]
    l4 = np.take_along_axis(logits, idx, axis=-1)
    w = np.exp(l4 - l4.max(-1, keepdims=True))
    w = w / w.sum(-1, keepdims=True)
    return idx.astype(np.int32), w.astype(np.float32)


def _build_nc(Cs, slices_list, repeat=1, mmdt=None):
    """Build the SPMD Bass program: 2 experts/core, SwiGLU over [H,C] tokens.

    Cs/slices_list: per-slot token capacity and PSUM n-slicing. Slot 0 holds
    the big-count experts, slot 1 the small ones (fewer/larger matmuls)."""
    mmdt = mmdt or MMDT
    CA = Cs[0]
    nc = bacc.Bacc(
        "TRN2",
        target_bir_lowering=False,
        debug=False,
        enable_asserts=True,
        num_devices=NCORES,
    )
    xt_d = nc.dram_tensor("xt", [EPC, H, CA], mmdt, kind="ExternalInput").ap()
    wg_d = nc.dram_tensor("wg", [EPC, H, I], mmdt, kind="ExternalInput").ap()
    wu_d = nc.dram_tensor("wu", [EPC, H, I], mmdt, kind="ExternalInput").ap()
    wd_d = nc.dram_tensor("wd", [EPC, I, H], mmdt, kind="ExternalInput").ap()
    yt_d = nc.dram_tensor("yt", [EPC, H, CA], F32, kind="ExternalOutput").ap()

    # k-tiles per batched weight DMA (SBUF budget: bf16 fits full-k blocks)
    WKB = KH if mmdt == BF16 else 8
    WKBD = KI if mmdt == BF16 else 7

    with tile.TileContext(nc) as tc, ExitStack() as ctx:
        xpool = ctx.enter_context(tc.tile_pool(name="x", bufs=2 if mmdt == BF16 else 1))
        apool = ctx.enter_context(tc.tile_pool(name="a", bufs=KI))
        wpool = ctx.enter_context(tc.tile_pool(name="w", bufs=4))
        tpool = ctx.enter_context(tc.tile_pool(name="t", bufs=4))
        ypool = ctx.enter_context(tc.tile_pool(name="y", bufs=3))
        ppool = ctx.enter_context(tc.tile_pool(name="p", bufs=2, space="PSUM"))

        def load_w(src_j, k0, nk, col0, tag):
            """One DMA: weight block [128, nk(k-tiles), 256(2 m-tiles)]."""
            t = wpool.tile([128, nk * 256], mmdt, tag=tag, name=tag)
            nc.sync.dma_start(
                t[:].rearrange("p (k c) -> p k c", c=256),
                src_j.rearrange("(k p) c -> p k c", p=128)[
                    :, k0 : k0 + nk, col0 : col0 + 256
                ],
            )
            return t

        def mmacc(psums, wt, k0, nk, rhs_of_k, ktot, slices):
            for kk in range(nk):
                k = k0 + kk
                for mi in range(2):
                    off = 0
                    for si, s in enumerate(slices):
                        nc.tensor.matmul(
                            psums[mi][si][:],
                            wt[:, (kk * 2 + mi) * 128 : (kk * 2 + mi + 1) * 128],
                            rhs_of_k(k)[:, off : off + s],
                            start=(k == 0),
                            stop=(k == ktot - 1),
                        )
                        off += s

        def psum_pair(slices):
            return [
                [ppool.tile([128, s], F32, tag=f"p{mi}{si}", name=f"p{mi}{si}")
                 for si, s in enumerate(slices)]
                for mi in range(2)
            ]

        def body():
            for j in range(EPC):
                C, slices = Cs[j], slices_list[j]
                # activations X^T for this expert: 4 chunked DMAs so the
                # first matmuls start after 1/4 of the load (parallel queues)
                xs = xpool.tile([128, KH * C], mmdt, tag="xk", name="xk")
                xt_r = xt_d[j].rearrange("(k p) c -> p k c", p=128)
                for k0 in range(0, KH, 4):
                    nc.sync.dma_start(
                        xs[:, k0 * C : (k0 + 4) * C].rearrange(
                            "p (k c) -> p k c", c=C),
                        xt_r[:, k0 : k0 + 4, :C],
                    )

                def xk(k):
                    return xs[:, k * C : (k + 1) * C]

                at = [apool.tile([128, C], mmdt, tag="ak", name="ak")
                      for _ in range(KI)]

                def atk(k):
                    return at[k][:]

                # ---- gate/up + SwiGLU, two I-tiles (m) at a time ----
                for mg in range(0, KI, 2):
                    col0 = mg * 128
                    pg = psum_pair(slices)
                    for k0 in range(0, KH, WKB):
                        wt = load_w(wg_d[j], k0, min(WKB, KH - k0), col0, "wg")
                        mmacc(pg, wt, k0, min(WKB, KH - k0), xk, KH, slices)
                    tg = [tpool.tile([128, C], F32, tag="tg", name="tg")
                          for _ in range(2)]
                    for mi in range(2):
                        off = 0
                        for si, s in enumerate(slices):
                            nc.scalar.activation(
                                tg[mi][:, off : off + s], pg[mi][si][:], SILU)
                            off += s
                    pu = psum_pair(slices)
                    for k0 in range(0, KH, WKB):
                        wt = load_w(wu_d[j], k0, min(WKB, KH - k0), col0, "wu")
                        mmacc(pu, wt, k0, min(WKB, KH - k0), xk, KH, slices)
                    # act = silu(g) * u
                    for mi in range(2):
                        off = 0
                        for si, s in enumerate(slices):
                            nc.vector.tensor_tensor(
                                at[mg + mi][:, off : off + s],
                                tg[mi][:, off : off + s],
                                pu[mi][si][:],
                                MULT,
                            )
                            off += s

                # ---- down projection, two H-tiles at a time ----
                for hg in range(0, KH, 2):
                    col0 = hg * 128
                    py = psum_pair(slices)
                    for k0 in range(0, KI, WKBD):
                        wt = load_w(wd_d[j], k0, min(WKBD, KI - k0), col0, "wd")
                        mmacc(py, wt, k0, min(WKBD, KI - k0), atk, KI, slices)
                    yo = ypool.tile([128, 2 * C], F32, tag="yo", name="yo")
                    for mi in range(2):
                        off = 0
                        for si, s in enumerate(slices):
                            nc.vector.tensor_copy(
                                yo[:, mi * C + off : mi * C + off + s],
                                py[mi][si][:])
                            off += s
                    nc.sync.dma_start(
                        yt_d[j].rearrange("(g p) c -> p g c", p=128)[
                            :, hg : hg + 2, :C],
                        yo[:].rearrange("p (g c) -> p g c", c=C),
                    )

        if repeat > 1:
            # HW loop used only by the timing harness: repeats the identical
            # body so HW exec time dominates the per-call dispatch overhead.
            with tc.For_i(0, repeat, 1):
                body()
        else:
            body()

    nc.compile()
    return nc


def _np_dt(mmdt):
    if mmdt == BF16:
        import ml_dtypes

        return ml_dtypes.bfloat16
    return np.float32


def _plan(counts):
    """Assign experts to (core, slot): slot 0 = 8 largest counts, slot 1 = 8
    smallest. Returns expert order and per-slot capacities."""
    order = np.argsort(-counts, kind="stable")
    caps = []
    for j in range(EPC):
        grp = order[j * NCORES : (j + 1) * NCORES]
        caps.append(max(256, int(-(-counts[grp].max() // 64) * 64)))
    return order, caps


def _prep(x, gate_proj, up_proj, down_proj, idx, order, caps, mmdt=None):
    """Gather per-expert token sets into per-core device inputs."""
    ndt = _np_dt(mmdt or MMDT)
    CA = caps[0]
    xf = np.ascontiguousarray(x.reshape(T, H).astype(np.float32))
    tok = [np.nonzero((idx == e).any(-1))[0] for e in range(E)]
    in_maps = []
    for c in range(NCORES):
        xt = np.zeros((EPC, H, CA), ndt)
        es = [int(order[j * NCORES + c]) for j in range(EPC)]
        for j, e in enumerate(es):
            te = tok[e]
            xt[j, :, : len(te)] = xf[te].T.astype(ndt)
        in_maps.append(
            {
                "xt": xt,
                "wg": np.ascontiguousarray(gate_proj[es]).astype(ndt),
                "wu": np.ascontiguousarray(up_proj[es]).astype(ndt),
                "wd": np.ascontiguousarray(down_proj[es]).astype(ndt),
            }
        )
    return in_maps, tok


def _combine(results, tok, idx, wts, order):
    """Weighted scatter-add of per-expert outputs back to [T, H]."""
    out = np.zeros((T, H), np.float64)
    for r in range(E):
        e = int(order[r])
        j, c = divmod(r, NCORES)
        yt = results[c]["yt"][j]  # [H, CA]
        te = tok[e]
        k = np.argmax(idx[te] == e, axis=-1)
        w = wts[te, k]
        out[te] += yt[:, : len(te)].T.astype(np.float64) * w[:, None]
    return out.astype(np.float32).reshape(B, S, H)


def kernel(x, router_w, expert_bias, gate_proj, up_proj, down_proj):
    x = np.asarray(x)
    idx, wts = _route(np.asarray(x), np.asarray(router_w), np.asarray(expert_bias))
    counts = np.bincount(idx.ravel(), minlength=E)
    order, caps = _plan(counts)
    nc = _build_nc(caps, [_slices(c) for c in caps])
    in_maps, tok = _prep(
        x, np.asarray(gate_proj), np.asarray(up_proj), np.asarray(down_proj),
        idx, order, caps,
    )
    res = run_bass_kernel_spmd(nc, in_maps, list(range(NCORES)))
    return _combine(res.results, tok, idx, wts, order)


# revision 21
# speedup vs baseline: 675.9060x; 1.0248x over previous
"""MoE ExpertBlock (16 experts, top-4, SwiGLU) on 8 Trainium2 NeuronCores.

Strategy (expert-parallel, per sharding hint):
  - Host: router (x @ router_w.T + bias -> softmax -> top-4) and token
    dispatch. This is ~0.07% of the model FLOPs.
  - Device: each of the 8 cores runs the SwiGLU FFN for 2 experts over the
    tokens routed to them. Experts are load-sorted: slot 0 = the 8 largest
    token counts (capacity CA), slot 1 = the 8 smallest (CB; when CB <= 512
    each PSUM tile is a single full-rate N=512 matmul).
    Feature-major layout ([H, C] activations, features on partitions) so the
    whole FFN chain needs zero on-device transposes; matmuls run in float32r
    (full PE rate for N>=256, ~2.6e-4 end-to-end rel-err vs the fp32
    reference). Weights stream from HBM in batched 3D-AP DMAs (~1-2 MB each,
    4-deep prefetch) -- small per-(k,m) weight DMAs were queue-bound.
  - Host: scatter-add the weighted per-expert outputs back (top-4 combine).

Device compute per core: 2 experts x 3 matmuls x [C,2048]x[2048,1792]-class
GEMMs ~ 24 GFLOP, vs 4x more for the dense-all-experts reference.
Measured ~420-460 us/core HW exec vs ~305 us pure PE-streaming bound.
"""

import sys

sys.path.insert(0, "/opt/trn_rl_repo")

from contextlib import ExitStack

import numpy as np

import concourse.bacc as bacc
import concourse.mybir as mybir
import concourse.tile as tile
from concourse.bass_utils import run_bass_kernel_spmd

B, S, H, I, E, TOPK = 2, 1024, 2048, 1792, 16, 4
T = B * S
NCORES = 8
EPC = E // NCORES  # experts per core
KH = H // 128  # 16 k-tiles over hidden dim
KI = I // 128  # 14 tiles over intermediate dim

F32 = mybir.dt.float32
F32R = mybir.dt.float32r
BF16 = mybir.dt.bfloat16
MMDT = F32R  # matmul operand dtype
WCFG = (8, 7, 4)  # (gate/up k-tiles per weight DMA, down k-tiles, wpool bufs)
FAST_START = False  # finer first xs/weight DMAs: HW-neutral, off for max-validated path
LOOP_HINTS = True  # prefetch loop-start IRAM blocks at the timing-loop back-edge
MULT = mybir.AluOpType.mult
SILU = mybir.ActivationFunctionType.Silu


def _slices(C):
    """Split C into contiguous chunks, each <=512 and >=256 (PSUM-bank sized,
    full-rate fp32r). C must be a multiple of 64 and >= 256."""
    n = -(-C // 512)
    out = []
    rem = C
    for i in range(n):
        s = min(512, -(-rem // (n - i) // 64) * 64)
        out.append(s)
        rem -= s
    assert rem == 0 and all(256 <= s <= 512 for s in out), (C, out)
    return out


def _route(x, router_w, expert_bias):
    """Host router: top-4 expert ids + renormalized weights per token."""
    xf = x.reshape(T, H).astype(np.float32)
    logits = xf @ router_w.T.astype(np.float32) + expert_bias.astype(np.float32)
    # top-4 by logit (same order as softmax); stable sort matches jax top_k ties
    idx = np.argsort(-logits, axis=-1, kind="stable")[:, :TOPK]
    l4 = np.take_along_axis(logits, idx, axis=-1)
    w = np.exp(l4 - l4.max(-1, keepdims=True))
    w = w / w.sum(-1, keepdims=True)
    return idx.astype(np.int32), w.astype(np.float32)


def _build_nc(Cs, slices_list, repeat=1, mmdt=None):
    """Build the SPMD Bass program: 2 experts/core, SwiGLU over [H,C] tokens.

    Cs/slices_list: per-slot token capacity and PSUM n-slicing. Slot 0 holds
    the big-count experts, slot 1 the small ones (fewer/larger matmuls)."""
    mmdt = mmdt or MMDT
    CA = Cs[0]
    nc = bacc.Bacc(
        "TRN2",
        target_bir_lowering=False,
        debug=False,
        enable_asserts=True,
        num_devices=NCORES,
    )
    xt_d = nc.dram_tensor("xt", [EPC, H, CA], mmdt, kind="ExternalInput").ap()
    wg_d = nc.dram_tensor("wg", [EPC, H, I], mmdt, kind="ExternalInput").ap()
    wu_d = nc.dram_tensor("wu", [EPC, H, I], mmdt, kind="ExternalInput").ap()
    wd_d = nc.dram_tensor("wd", [EPC, I, H], mmdt, kind="ExternalInput").ap()
    yt_d = nc.dram_tensor("yt", [EPC, H, CA], F32, kind="ExternalOutput").ap()

    # k-tiles per batched weight DMA (SBUF budget: bf16 fits full-k blocks)
    WKB = KH if mmdt == BF16 else WCFG[0]
    WKBD = KI if mmdt == BF16 else WCFG[1]

    with tile.TileContext(nc) as tc, ExitStack() as ctx:
        xpool = ctx.enter_context(tc.tile_pool(name="x", bufs=2 if mmdt == BF16 else 1))
        apool = ctx.enter_context(tc.tile_pool(name="a", bufs=KI))
        wpool = ctx.enter_context(tc.tile_pool(name="w", bufs=WCFG[2]))
        tpool = ctx.enter_context(tc.tile_pool(name="t", bufs=4))
        ypool = ctx.enter_context(tc.tile_pool(name="y", bufs=3))
        ppool = ctx.enter_context(tc.tile_pool(name="p", bufs=2, space="PSUM"))

        def load_w(src_j, k0, nk, col0, tag):
            """One DMA: weight block [128, nk(k-tiles), 256(2 m-tiles)]."""
            t = wpool.tile([128, nk * 256], mmdt, tag=tag, name=tag)
            nc.sync.dma_start(
                t[:].rearrange("p (k c) -> p k c", c=256),
                src_j.rearrange("(k p) c -> p k c", p=128)[
                    :, k0 : k0 + nk, col0 : col0 + 256
                ],
            )
            return t

        def mmacc(psums, wt, k0, nk, rhs_of_k, ktot, slices):
            for kk in range(nk):
                k = k0 + kk
                for mi in range(2):
                    off = 0
                    for si, s in enumerate(slices):
                        nc.tensor.matmul(
                            psums[mi][si][:],
                            wt[:, (kk * 2 + mi) * 128 : (kk * 2 + mi + 1) * 128],
                            rhs_of_k(k)[:, off : off + s],
                            start=(k == 0),
                            stop=(k == ktot - 1),
                        )
                        off += s

        def psum_pair(slices):
            return [
                [ppool.tile([128, s], F32, tag=f"p{mi}{si}", name=f"p{mi}{si}")
                 for si, s in enumerate(slices)]
                for mi in range(2)
            ]

        def body():
            for j in range(EPC):
                C, slices = Cs[j], slices_list[j]
                # activations X^T for this expert: 4 chunked DMAs so the
                # first matmuls start after 1/4 of the load (parallel queues)
                xs = xpool.tile([128, KH * C], mmdt, tag="xk", name="xk")
                xt_r = xt_d[j].rearrange("(k p) c -> p k c", p=128)
                if FAST_START and j == 0:
                    xchunks = [(0, 1), (1, 1), (2, 2), (4, 4), (8, 4), (12, 4)]
                else:
                    xchunks = [(k0, 4) for k0 in range(0, KH, 4)]
                for k0, nk in xchunks:
                    nc.sync.dma_start(
                        xs[:, k0 * C : (k0 + nk) * C].rearrange(
                            "p (k c) -> p k c", c=C),
                        xt_r[:, k0 : k0 + nk, :C],
                    )

                def xk(k):
                    return xs[:, k * C : (k + 1) * C]

                at = [apool.tile([128, C], mmdt, tag="ak", name="ak")
                      for _ in range(KI)]

                def atk(k):
                    return at[k][:]

                # ---- gate/up + SwiGLU, two I-tiles (m) at a time ----
                for mg in range(0, KI, 2):
                    col0 = mg * 128
                    pg = psum_pair(slices)
                    if FAST_START and j == 0 and mg == 0:
                        kblocks = [(0, 2), (2, 6), (8, 8)]
                    else:
                        kblocks = [(k0, min(WKB, KH - k0))
                                   for k0 in range(0, KH, WKB)]
                    for k0, nk in kblocks:
                        wt = load_w(wg_d[j], k0, nk, col0, "wg")
                        mmacc(pg, wt, k0, nk, xk, KH, slices)
                    tg = [tpool.tile([128, C], F32, tag="tg", name="tg")
                          for _ in range(2)]
                    for mi in range(2):
                        off = 0
                        for si, s in enumerate(slices):
                            nc.scalar.activation(
                                tg[mi][:, off : off + s], pg[mi][si][:], SILU)
                            off += s
                    pu = psum_pair(slices)
                    for k0 in range(0, KH, WKB):
                        wt = load_w(wu_d[j], k0, min(WKB, KH - k0), col0, "wu")
                        mmacc(pu, wt, k0, min(WKB, KH - k0), xk, KH, slices)
                    # act = silu(g) * u
                    for mi in range(2):
                        off = 0
                        for si, s in enumerate(slices):
                            nc.vector.tensor_tensor(
                                at[mg + mi][:, off : off + s],
                                tg[mi][:, off : off + s],
                                pu[mi][si][:],
                                MULT,
                            )
                            off += s

                # ---- down projection, two H-tiles at a time ----
                for hg in range(0, KH, 2):
                    col0 = hg * 128
                    py = psum_pair(slices)
                    for k0 in range(0, KI, WKBD):
                        wt = load_w(wd_d[j], k0, min(WKBD, KI - k0), col0, "wd")
                        mmacc(py, wt, k0, min(WKBD, KI - k0), atk, KI, slices)
                    yo = ypool.tile([128, 2 * C], F32, tag="yo", name="yo")
                    for mi in range(2):
                        off = 0
                        for si, s in enumerate(slices):
                            nc.vector.tensor_copy(
                                yo[:, mi * C + off : mi * C + off + s],
                                py[mi][si][:])
                            off += s
                    nc.sync.dma_start(
                        yt_d[j].rearrange("(g p) c -> p g c", p=128)[
                            :, hg : hg + 2, :C],
                        yo[:].rearrange("p (g c) -> p g c", c=C),
                    )

        if repeat > 1:
            # HW loop used only by the timing harness: repeats the identical
            # body so HW exec time dominates the per-call dispatch overhead.
            hints = (
                (mybir.EngineType.PE, mybir.EngineType.SP) if LOOP_HINTS else ()
            )
            with tc.For_i(0, repeat, 1, hint_engines=hints):
                body()
        else:
            body()

    nc.compile()
    return nc


def _np_dt(mmdt):
    if mmdt == BF16:
        import ml_dtypes

        return ml_dtypes.bfloat16
    return np.float32


def _plan(counts):
    """Assign experts to (core, slot): slot 0 = 8 largest counts, slot 1 = 8
    smallest. Returns expert order and per-slot capacities."""
    order = np.argsort(-counts, kind="stable")
    caps = []
    for j in range(EPC):
        grp = order[j * NCORES : (j + 1) * NCORES]
        caps.append(max(256, int(-(-counts[grp].max() // 64) * 64)))
    return order, caps


def _prep(x, gate_proj, up_proj, down_proj, idx, order, caps, mmdt=None):
    """Gather per-expert token sets into per-core device inputs."""
    ndt = _np_dt(mmdt or MMDT)
    CA = caps[0]
    xf = np.ascontiguousarray(x.reshape(T, H).astype(np.float32))
    tok = [np.nonzero((idx == e).any(-1))[0] for e in range(E)]
    in_maps = []
    for c in range(NCORES):
        xt = np.zeros((EPC, H, CA), ndt)
        es = [int(order[j * NCORES + c]) for j in range(EPC)]
        for j, e in enumerate(es):
            te = tok[e]
            xt[j, :, : len(te)] = xf[te].T.astype(ndt)
        in_maps.append(
            {
                "xt": xt,
                "wg": np.ascontiguousarray(gate_proj[es]).astype(ndt),
                "wu": np.ascontiguousarray(up_proj[es]).astype(ndt),
                "wd": np.ascontiguousarray(down_proj[es]).astype(ndt),
            }
        )
    return in_maps, tok


def _combine(results, tok, idx, wts, order):
    """Weighted scatter-add of per-expert outputs back to [T, H]."""
    out = np.zeros((T, H), np.float64)
    for r in range(E):
        e = int(order[r])
        j, c = divmod(r, NCORES)
        yt = results[c]["yt"][j]  # [H, CA]
        te = tok[e]
        k = np.argmax(idx[te] == e, axis=-1)
        w = wts[te, k]
        out[te] += yt[:, : len(te)].T.astype(np.float64) * w[:, None]
    return out.astype(np.float32).reshape(B, S, H)


def _spot_check(results, tok, order, xf, gate_proj, up_proj, down_proj):
    """Exact host recompute of sampled token rows per expert. Catches the
    (rare, transient) corrupted-execution failure mode observed once on this
    hardware; fp32r disagreement is ~3e-4, corruption is ~5e-2."""
    rng = np.random.default_rng(0)
    for r in range(E):
        e = int(order[r])
        j, c = divmod(r, NCORES)
        te = tok[e]
        if len(te) == 0:
            continue
        pick = rng.choice(len(te), size=min(48, len(te)), replace=False)
        xs = xf[te[pick]].astype(np.float64)
        g = xs @ gate_proj[e].astype(np.float64)
        u = xs @ up_proj[e].astype(np.float64)
        act = g / (1.0 + np.exp(-g)) * u
        y = act @ down_proj[e].astype(np.float64)
        got = results[c]["yt"][j][:, pick].T.astype(np.float64)
        rel = np.abs(got - y).max() / max(np.abs(y).max(), 1e-6)
        if rel > 5e-3:
            return False
    return True


def kernel(x, router_w, expert_bias, gate_proj, up_proj, down_proj):
    x = np.asarray(x)
    gate_proj = np.asarray(gate_proj)
    up_proj = np.asarray(up_proj)
    down_proj = np.asarray(down_proj)
    idx, wts = _route(x, np.asarray(router_w), np.asarray(expert_bias))
    counts = np.bincount(idx.ravel(), minlength=E)
    order, caps = _plan(counts)
    nc = _build_nc(caps, [_slices(c) for c in caps])
    in_maps, tok = _prep(x, gate_proj, up_proj, down_proj, idx, order, caps)
    xf = np.ascontiguousarray(x.reshape(T, H).astype(np.float32))
    res = run_bass_kernel_spmd(nc, in_maps, list(range(NCORES)))
    for _ in range(2):
        if _spot_check(res.results, tok, order, xf, gate_proj, up_proj,
                       down_proj):
            break
        res = run_bass_kernel_spmd(nc, in_maps, list(range(NCORES)))
    return _combine(res.results, tok, idx, wts, order)
